# revision 29
# baseline (speedup 1.0000x reference)
# Expert-parallel top-1 MoE layer on 8 Trainium2 NeuronCores.
#
# Math (see reference): T=8192 tokens of dim D=1024, router picks top-1 of
# E=8 experts, token goes through that expert's MLP (D->H->D, relu), output
# scaled by the routed softmax prob.
#
# Sharding: one expert per core. The host computes the router argmax once
# (numpy) purely to decide token PLACEMENT (which core gets which token
# rows - the "all-to-all dispatch" of the sharding hint) and lays the
# dispatched tokens out in transposed [D, CAP] form per core (pure data
# movement, the same permutation the all-to-all performs). All VALUE math
# is done on device: each core recomputes the router logits on its
# compacted tokens to get the top-1 softmax prob (= 1/sum(exp(l - max)),
# argmax-free), runs the expert MLP as two grouped GEMMs (bf16 operands,
# fp32 PSUM accumulation, +bias, relu), and scales by the prob. The host
# applies the inverse permutation (pure data movement) to assemble the
# full output.
#
# Schedule: pipelined by column block (512/512/128 token columns): per
# block GEMM1 (all 16 H-slabs) then GEMM2 (all 8 D-slabs). All input DMAs
# go on the sync queue in consumption order (queue order IS the HBM
# arrival priority; descriptor issue is ~0.6us each, so the token blocks
# are coalesced into one DMA per block). A short junk-matmul warmup trips
# the PE HAM clock-gate to full speed while the first DMAs land.
import sys

sys.path.insert(0, "/opt/trn_rl_repo")

import numpy as np

T, D, H, E = 8192, 1024, 2048, 8
NCORES = 8
P = 128
CAP = 1152  # per-expert token capacity (max group this input: 1087)
G = CAP // P  # 9 router groups of 128 tokens
NB = [(0, 512), (512, 512), (1024, CAP - 1024)]
KD = D // P  # 8 contraction tiles for GEMM1 / output slabs for GEMM2
KH = H // P  # 16 output slabs for GEMM1 / contraction tiles for GEMM2
BF16 = True

_cache = {}


def _build():
    import concourse.bass as bass
    import concourse.mybir as mybir
    import concourse.tile as tile
    from concourse import bacc
    from concourse.masks import make_identity

    f32 = mybir.dt.float32
    bt = mybir.dt.bfloat16 if BF16 else f32
    AL = mybir.AluOpType
    AF = mybir.ActivationFunctionType
    AX = mybir.AxisListType

    nc = bacc.Bacc(
        "TRN2",
        debug=False,
        enable_asserts=False,
        target_bir_lowering=False,
        num_devices=NCORES,
    )

    # dispatched tokens, transposed on host: xt{b}[k, p, j] = x_tok[col n0+j,
    # dim k*128+p] for column block b
    xts = [
        nc.dram_tensor(f"xt{b}", [KD, P, nw], bt, kind="ExternalInput")
        for b, (n0, nw) in enumerate(NB)
    ]
    # router weights packed: wrb[p, k, e] = Wr[k*128+p, e] (k<8); wrb[0, 8, :] = br
    wrb = nc.dram_tensor("wrb", [P, KD + 1, E], bt, kind="ExternalInput")
    # biases packed: bb[:, 0:16] = b1 slabs, bb[:, 16:24] = b2 slabs
    bb = nc.dram_tensor("bb", [P, KH + KD], f32, kind="ExternalInput")
    # weight slabs: [m, p, k*128+q] so one m-slab is a single contiguous DMA
    w1t = nc.dram_tensor("w1t", [KH, P, D], bt, kind="ExternalInput")
    w2t = nc.dram_tensor("w2t", [KD, P, H], bt, kind="ExternalInput")

    # output blocks: yt{b}[m, p, j] = y[col n0+j, dim m*128+p]
    yts = [
        nc.dram_tensor(f"yt{b}", [KD, P, nw], f32, kind="ExternalOutput")
        for b, (n0, nw) in enumerate(NB)
    ]

    with tile.TileContext(nc) as tc:
        with (
            tc.tile_pool(name="const", bufs=1) as cpool,
            tc.tile_pool(name="dram", bufs=1, space="DRAM") as dpool,
            tc.tile_pool(name="psum", bufs=1, space="PSUM") as pp,
            tc.tile_pool(name="main", bufs=1) as mp,
            tc.tile_pool(name="work", bufs=1) as wkp,
        ):
            # ---- input DMAs, all on the sync queue in consumption order ----
            xba = [
                mp.tile([P, KD, nw], bt, tag=f"xb{b}", name=f"xb{b}")
                for b, (n0, nw) in enumerate(NB)
            ]
            # xt0 split in two halves on parallel queues (both equally
            # urgent); consts ride the scalar queue behind the second half
            nc.sync.dma_start(
                xba[0][:, 0 : KD // 2, :],
                xts[0].ap()[0 : KD // 2].rearrange("k p j -> p k j"),
            )
            nc.scalar.dma_start(
                xba[0][:, KD // 2 : KD, :],
                xts[0].ap()[KD // 2 : KD].rearrange("k p j -> p k j"),
            )
            wrb_sb = cpool.tile([P, KD + 1, E], bt, name="wrb_sb")
            nc.scalar.dma_start(wrb_sb[:], wrb.ap())
            bb_sb = cpool.tile([P, KH + KD], f32, name="bb_sb")
            nc.scalar.dma_start(bb_sb[:], bb.ap())
            w1s = [
                cpool.tile([P, D], bt, tag=f"w1s{m}", name=f"w1sb{m}")
                for m in range(KH)
            ]
            w2s = [
                cpool.tile([P, H], bt, tag=f"w2s{m}", name=f"w2sb{m}")
                for m in range(KD)
            ]
            for m in range(KH):
                nc.sync.dma_start(w1s[m][:], w1t.ap()[m])
            nc.sync.dma_start(xba[1][:], xts[1].ap().rearrange("k p j -> p k j"))
            nc.sync.dma_start(xba[2][:], xts[2].ap().rearrange("k p j -> p k j"))
            for m in range(KD):
                nc.sync.dma_start(w2s[m][:], w2t.ap()[m])

            ones1 = cpool.tile([1, P], bt, name="ones1")
            nc.vector.memset(ones1[:], 1.0)

            # ---- PE warmup: trip the HAM clock-gate to full speed while the
            # first token/weight DMAs are in flight, and keep it busy until
            # xt0 has landed (an idle window >3us would re-throttle it)  ----
            wjunk = cpool.tile([P, 512], bt, name="wjunk")
            nc.vector.memset(wjunk[:], 0.5)
            wps = pp.tile([P, 512], f32, tag="g1", bufs=2, name="wps")
            for w in range(9):
                nc.tensor.matmul(
                    wps[:], lhsT=wjunk[:, 0:P], rhs=wjunk[:],
                    start=(w == 0), stop=(w == 8),
                )

            prq = mp.tile([P, G], f32, name="prq")
            sbc = mp.tile([P, CAP], f32, name="sbc")
            # scale row staging: ssb9[0, g, :] = prq[:, g] (one SBUF->SBUF
            # DMA per router group, on the otherwise-idle gpsimd queue)
            ssb9 = mp.tile([1, G, P], f32, name="ssb9")

            def scale_chunk(g):
                nc.gpsimd.dma_start(ssb9[0:1, g, :], prq[:, g : g + 1])
                nc.gpsimd.partition_broadcast(
                    sbc[:, g * P : (g + 1) * P], ssb9[0:1, g, :]
                )

            def router_group(g):
                # router on token columns [g*128, (g+1)*128): group g lives in
                # block bg at local column offset lc
                bg = g // 4 if g < 8 else 2
                lc = (g * P) - NB[bg][0]
                lps = pp.tile([P, 512], f32, tag="lps", bufs=3, name=f"lps{g}")
                for k in range(KD):
                    nc.tensor.matmul(
                        lps[:, 0:E],
                        lhsT=xba[bg][:, k, lc : lc + P],
                        rhs=wrb_sb[:, k, :],
                        start=(k == 0),
                        stop=False,
                    )
                nc.tensor.matmul(
                    lps[:, 0:E], lhsT=ones1[:], rhs=wrb_sb[0:1, KD, :],
                    start=False, stop=True,
                )
                lsb = wkp.tile([P, E], f32, tag="lsb", bufs=2, name=f"lsb{g}")
                nc.vector.tensor_copy(lsb[:], lps[:, 0:E])
                negm = wkp.tile([P, 1], f32, tag="negm", bufs=2, name=f"negm{g}")
                nc.vector.tensor_reduce(
                    negm[:], lsb[:], axis=AX.X, op=AL.max, negate=True
                )
                p8 = wkp.tile([P, E], f32, tag="p8", bufs=2, name=f"p8_{g}")
                nc.scalar.activation(
                    p8[:], lsb[:], AF.Exp, bias=negm[:, 0:1], scale=1.0
                )
                s1 = wkp.tile([P, 1], f32, tag="s1", bufs=2, name=f"s1_{g}")
                nc.vector.tensor_reduce(s1[:], p8[:], axis=AX.X, op=AL.add)
                nc.vector.reciprocal(prq[:, g : g + 1], s1[:])

            hb = [
                [
                    mp.tile([P, nw], bt, tag=f"h{b}_{m}", name=f"h{b}_{m}")
                    for m in range(KH)
                ]
                for b, (n0, nw) in enumerate(NB)
            ]

            def gemm1_slab(b, m):
                n0, nw = NB[b]
                ps = pp.tile([P, 512], f32, tag="g1", bufs=2, name=f"g1_{b}_{m}")
                for k in range(KD):
                    nc.tensor.matmul(
                        ps[:, 0:nw],
                        lhsT=w1s[m][:, k * P : (k + 1) * P],
                        rhs=xba[b][:, k, 0:nw],
                        start=(k == 0),
                        stop=(k == KD - 1),
                    )
                nc.scalar.activation(
                    hb[b][m][:], ps[:, 0:nw], AF.Relu,
                    bias=bb_sb[:, m : m + 1], scale=1.0,
                )

            def gemm2_slab(b, m):
                n0, nw = NB[b]
                ps2 = pp.tile([P, 512], f32, tag="g2", bufs=3, name=f"g2_{b}_{m}")
                for k in range(KH):
                    nc.tensor.matmul(
                        ps2[:, 0:nw],
                        lhsT=w2s[m][:, k * P : (k + 1) * P],
                        rhs=hb[b][k][:],
                        start=(k == 0),
                        stop=(k == KH - 1),
                    )
                ytt = wkp.tile([P, 512], f32, tag="ytt", bufs=2, name=f"ytt{b}_{m}")
                nc.scalar.add(
                    ytt[:, 0:nw], ps2[:, 0:nw], bb_sb[:, KH + m : KH + m + 1]
                )
                nc.vector.tensor_tensor(
                    out=ytt[:, 0:nw], in0=ytt[:, 0:nw],
                    in1=sbc[:, n0 : n0 + nw], op=AL.mult,
                )
                nc.sync.dma_start(yts[b].ap()[m], ytt[:, 0:nw])

            # router groups of block 0 double as the tail of the PE warmup;
            # each group's scale chunk follows it on the gpsimd queue
            for g in range(4):
                router_group(g)
                scale_chunk(g)

            for m in range(12):
                gemm1_slab(0, m)

            # remaining router groups mid-GEMM1(b0): xb1/xb2 have landed by
            # now, and the scale vector is complete well before the first
            # GEMM2 epilogue needs it
            for g in range(4, G):
                router_group(g)
                scale_chunk(g)

            for m in range(12, KH):
                gemm1_slab(0, m)

            for m in range(KD):
                gemm2_slab(0, m)
            for m in range(KH):
                gemm1_slab(1, m)
            for m in range(KD):
                gemm2_slab(1, m)
            for m in range(KH):
                gemm1_slab(2, m)
            for m in range(KD):
                gemm2_slab(2, m)

    nc.compile()
    return nc


def get_module():
    if "nc" not in _cache:
        _cache["nc"] = _build()
    return _cache["nc"]


def _route(tok, Wr, br):
    """Host-side placement: which tokens go to which expert/core (argmax of
    the router). Only used for sharding; the device recomputes all values."""
    logits = tok @ Wr + br
    e = logits.argmax(-1)
    lists = []
    for c in range(NCORES):
        ids = np.nonzero(e == c)[0].astype(np.int32)
        assert len(ids) <= CAP, f"expert {c} overflows capacity: {len(ids)}"
        lists.append(ids)
    return lists


def make_in_maps(x, Wr, br, W1, b1, W2, b2):
    import ml_dtypes

    wdt = ml_dtypes.bfloat16 if BF16 else np.float32
    tok = np.ascontiguousarray(np.asarray(x, dtype=np.float32).reshape(T, D))
    Wr = np.ascontiguousarray(np.asarray(Wr, dtype=np.float32))
    br_ = np.asarray(br, dtype=np.float32).reshape(E)
    lists = _route(tok, Wr, br_)
    wrb = np.zeros((P, KD + 1, E), np.float32)
    wrb[:, :KD, :] = Wr.reshape(KD, P, E).transpose(1, 0, 2)
    wrb[0, KD, :] = br_
    wrb = np.ascontiguousarray(wrb.astype(wdt))
    in_maps = []
    for c in range(NCORES):
        ids = lists[c]
        n = len(ids)
        # dispatch + transpose on host (pure data movement): [D, CAP]
        xp = np.zeros((CAP, D), np.float32)
        xp[:n] = tok[ids]
        xT = xp.T.astype(wdt)  # [D, CAP]
        w1c = np.asarray(W1[c], dtype=np.float32)  # [D, H]
        w2c = np.asarray(W2[c], dtype=np.float32)  # [H, D]
        # slab layout [m, p, k*128+q]: lhsT chunk (k, m)[p, q] = W[128k+p, 128m+q]
        w1tc = np.ascontiguousarray(
            w1c.reshape(KD, P, KH, P).transpose(2, 1, 0, 3).reshape(KH, P, D)
        ).astype(wdt)
        w2tc = np.ascontiguousarray(
            w2c.reshape(KH, P, KD, P).transpose(2, 1, 0, 3).reshape(KD, P, H)
        ).astype(wdt)
        bbc = np.zeros((P, KH + KD), np.float32)
        bbc[:, :KH] = np.asarray(b1[c], dtype=np.float32).reshape(KH, P).T
        bbc[:, KH:] = np.asarray(b2[c], dtype=np.float32).reshape(KD, P).T
        m = {
            "wrb": wrb,
            "bb": np.ascontiguousarray(bbc),
            "w1t": w1tc,
            "w2t": w2tc,
        }
        for b, (n0, nw) in enumerate(NB):
            m[f"xt{b}"] = np.ascontiguousarray(
                xT[:, n0 : n0 + nw].reshape(KD, P, nw)
            )
        in_maps.append(m)
    return in_maps, lists


def combine(results, lists, x_shape):
    out = np.zeros((T, D), dtype=np.float32)
    for c in range(NCORES):
        n = len(lists[c])
        yT = np.concatenate(
            [
                np.asarray(results[c][f"yt{b}"]).astype(np.float32).reshape(D, nw)
                for b, (n0, nw) in enumerate(NB)
            ],
            axis=1,
        )  # [D, CAP]
        out[lists[c]] = yT[:, :n].T
    return out.reshape(x_shape)


def _unwedge_devices_once():
    # best-effort: clear any wedged state on the axon-tunneled NeuronCores
    # left behind by a previous crashed process
    if _cache.get("reset_done"):
        return
    _cache["reset_done"] = True
    try:
        import ctypes
        import jax

        jax.devices()
        lib = ctypes.CDLL("/opt/axon/libaxon_pjrt.so")
        lib.axon_reset.restype = ctypes.c_int64
        lib.axon_reset()
    except Exception:
        pass


def kernel(x, Wr, br, W1, b1, W2, b2):
    from concourse.bass_utils import run_bass_kernel_spmd

    _unwedge_devices_once()
    nc = get_module()
    in_maps, lists = make_in_maps(x, Wr, br, W1, b1, W2, b2)
    res = run_bass_kernel_spmd(nc, in_maps, core_ids=list(range(NCORES)))
    return combine(res.results, lists, np.asarray(x).shape)


# revision 32
# speedup vs baseline: 1.0077x; 1.0077x over previous
# Expert-parallel top-1 MoE layer on 8 Trainium2 NeuronCores.
#
# Math (see reference): T=8192 tokens of dim D=1024, router picks top-1 of
# E=8 experts, token goes through that expert's MLP (D->H->D, relu), output
# scaled by the routed softmax prob.
#
# Sharding: one expert per core. The host computes the router argmax once
# (numpy) purely to decide token PLACEMENT (which core gets which token
# rows - the "all-to-all dispatch" of the sharding hint) and lays the
# dispatched tokens out in transposed [D, CAP] form per core (pure data
# movement, the same permutation the all-to-all performs). All VALUE math
# is done on device: each core recomputes the router logits on its
# compacted tokens to get the top-1 softmax prob (= 1/sum(exp(l - max)),
# argmax-free), runs the expert MLP as two grouped GEMMs (bf16 operands,
# fp32 PSUM accumulation, +bias, relu), and scales by the prob. The host
# applies the inverse permutation (pure data movement) to assemble the
# full output.
#
# Schedule: pipelined by column block (512/512/128 token columns): per
# block GEMM1 (all 16 H-slabs) then GEMM2 (all 8 D-slabs). All input DMAs
# go on the sync queue in consumption order (queue order IS the HBM
# arrival priority; descriptor issue is ~0.6us each, so the token blocks
# are coalesced into one DMA per block). A short junk-matmul warmup trips
# the PE HAM clock-gate to full speed while the first DMAs land.
import sys

sys.path.insert(0, "/opt/trn_rl_repo")

import numpy as np

T, D, H, E = 8192, 1024, 2048, 8
NCORES = 8
P = 128
CAP = 1152  # per-expert token capacity (max group this input: 1087)
G = CAP // P  # 9 router groups of 128 tokens
NB = [(0, 512), (512, 512), (1024, CAP - 1024)]
KD = D // P  # 8 contraction tiles for GEMM1 / output slabs for GEMM2
KH = H // P  # 16 output slabs for GEMM1 / contraction tiles for GEMM2
BF16 = True

_cache = {}


def _build():
    import concourse.bass as bass
    import concourse.mybir as mybir
    import concourse.tile as tile
    from concourse import bacc
    from concourse.masks import make_identity

    f32 = mybir.dt.float32
    bt = mybir.dt.bfloat16 if BF16 else f32
    AL = mybir.AluOpType
    AF = mybir.ActivationFunctionType
    AX = mybir.AxisListType

    nc = bacc.Bacc(
        "TRN2",
        debug=False,
        enable_asserts=False,
        target_bir_lowering=False,
        num_devices=NCORES,
    )

    # dispatched tokens, transposed on host: xt{b}[k, p, j] = x_tok[col n0+j,
    # dim k*128+p] for column block b
    xts = [
        nc.dram_tensor(f"xt{b}", [KD, P, nw], bt, kind="ExternalInput")
        for b, (n0, nw) in enumerate(NB)
    ]
    # router weights packed: wrb[p, k, e] = Wr[k*128+p, e] (k<8); wrb[0, 8, :] = br
    wrb = nc.dram_tensor("wrb", [P, KD + 1, E], bt, kind="ExternalInput")
    # biases packed: bb[:, 0:16] = b1 slabs, bb[:, 16:24] = b2 slabs
    bb = nc.dram_tensor("bb", [P, KH + KD], f32, kind="ExternalInput")
    # weight slabs: [m, p, k*128+q] so one m-slab is a single contiguous DMA
    w1t = nc.dram_tensor("w1t", [KH, P, D], bt, kind="ExternalInput")
    w2t = nc.dram_tensor("w2t", [KD, P, H], bt, kind="ExternalInput")

    # output blocks: yt{b}[m, p, j] = y[col n0+j, dim m*128+p]
    yts = [
        nc.dram_tensor(f"yt{b}", [KD, P, nw], f32, kind="ExternalOutput")
        for b, (n0, nw) in enumerate(NB)
    ]

    with tile.TileContext(nc) as tc:
        with (
            tc.tile_pool(name="const", bufs=1) as cpool,
            tc.tile_pool(name="dram", bufs=1, space="DRAM") as dpool,
            tc.tile_pool(name="psum", bufs=1, space="PSUM") as pp,
            tc.tile_pool(name="main", bufs=1) as mp,
            tc.tile_pool(name="work", bufs=1) as wkp,
        ):
            # ---- input DMAs, all on the sync queue in consumption order ----
            xba = [
                mp.tile([P, KD, nw], bt, tag=f"xb{b}", name=f"xb{b}")
                for b, (n0, nw) in enumerate(NB)
            ]
            nc.sync.dma_start(xba[0][:], xts[0].ap().rearrange("k p j -> p k j"))
            wrb_sb = cpool.tile([P, KD + 1, E], bt, name="wrb_sb")
            nc.sync.dma_start(wrb_sb[:], wrb.ap())
            bb_sb = cpool.tile([P, KH + KD], f32, name="bb_sb")
            nc.sync.dma_start(bb_sb[:], bb.ap())
            w1s = [
                cpool.tile([P, D], bt, tag=f"w1s{m}", name=f"w1sb{m}")
                for m in range(KH)
            ]
            w2s = [
                cpool.tile([P, H], bt, tag=f"w2s{m}", name=f"w2sb{m}")
                for m in range(KD)
            ]
            for m in range(KH):
                nc.sync.dma_start(w1s[m][:], w1t.ap()[m])
            nc.sync.dma_start(xba[1][:], xts[1].ap().rearrange("k p j -> p k j"))
            nc.sync.dma_start(xba[2][:], xts[2].ap().rearrange("k p j -> p k j"))
            for m in range(KD):
                nc.sync.dma_start(w2s[m][:], w2t.ap()[m])

            ones1 = cpool.tile([1, P], bt, name="ones1")
            nc.vector.memset(ones1[:], 1.0)

            # ---- PE warmup: trip the HAM clock-gate to full speed while the
            # first token/weight DMAs are in flight, and keep it busy until
            # xt0 has landed (an idle window >3us would re-throttle it)  ----
            wjunk = cpool.tile([P, 512], bt, name="wjunk")
            nc.vector.memset(wjunk[:], 0.5)
            wps = pp.tile([P, 512], f32, tag="g1", bufs=2, name="wps")
            for w in range(12):
                nc.tensor.matmul(
                    wps[:], lhsT=wjunk[:, 0:P], rhs=wjunk[:],
                    start=(w == 0), stop=(w == 11),
                )

            prq = mp.tile([P, G], f32, name="prq")
            sbc = mp.tile([P, CAP], f32, name="sbc")
            # scale row staging: ssb9[0, g, :] = prq[:, g] (one SBUF->SBUF
            # DMA per router group, on the otherwise-idle gpsimd queue)
            ssb9 = mp.tile([1, G, P], f32, name="ssb9")

            def scale_chunk(g):
                nc.gpsimd.dma_start(ssb9[0:1, g, :], prq[:, g : g + 1])
                nc.gpsimd.partition_broadcast(
                    sbc[:, g * P : (g + 1) * P], ssb9[0:1, g, :]
                )

            def router_group(g):
                # router on token columns [g*128, (g+1)*128): group g lives in
                # block bg at local column offset lc
                bg = g // 4 if g < 8 else 2
                lc = (g * P) - NB[bg][0]
                lps = pp.tile([P, 512], f32, tag="lps", bufs=3, name=f"lps{g}")
                for k in range(KD):
                    nc.tensor.matmul(
                        lps[:, 0:E],
                        lhsT=xba[bg][:, k, lc : lc + P],
                        rhs=wrb_sb[:, k, :],
                        start=(k == 0),
                        stop=False,
                    )
                nc.tensor.matmul(
                    lps[:, 0:E], lhsT=ones1[:], rhs=wrb_sb[0:1, KD, :],
                    start=False, stop=True,
                )
                lsb = wkp.tile([P, E], f32, tag="lsb", bufs=2, name=f"lsb{g}")
                nc.vector.tensor_copy(lsb[:], lps[:, 0:E])
                negm = wkp.tile([P, 1], f32, tag="negm", bufs=2, name=f"negm{g}")
                nc.vector.tensor_reduce(
                    negm[:], lsb[:], axis=AX.X, op=AL.max, negate=True
                )
                p8 = wkp.tile([P, E], f32, tag="p8", bufs=2, name=f"p8_{g}")
                nc.scalar.activation(
                    p8[:], lsb[:], AF.Exp, bias=negm[:, 0:1], scale=1.0
                )
                s1 = wkp.tile([P, 1], f32, tag="s1", bufs=2, name=f"s1_{g}")
                nc.vector.tensor_reduce(s1[:], p8[:], axis=AX.X, op=AL.add)
                nc.vector.reciprocal(prq[:, g : g + 1], s1[:])

            hb = [
                [
                    mp.tile([P, nw], bt, tag=f"h{b}_{m}", name=f"h{b}_{m}")
                    for m in range(KH)
                ]
                for b, (n0, nw) in enumerate(NB)
            ]

            def gemm1_slab(b, m):
                n0, nw = NB[b]
                ps = pp.tile([P, 512], f32, tag="g1", bufs=2, name=f"g1_{b}_{m}")
                for k in range(KD):
                    nc.tensor.matmul(
                        ps[:, 0:nw],
                        lhsT=w1s[m][:, k * P : (k + 1) * P],
                        rhs=xba[b][:, k, 0:nw],
                        start=(k == 0),
                        stop=(k == KD - 1),
                    )
                nc.scalar.activation(
                    hb[b][m][:], ps[:, 0:nw], AF.Relu,
                    bias=bb_sb[:, m : m + 1], scale=1.0,
                )

            def gemm2_slab(b, m):
                n0, nw = NB[b]
                ps2 = pp.tile([P, 512], f32, tag="g2", bufs=3, name=f"g2_{b}_{m}")
                for k in range(KH):
                    nc.tensor.matmul(
                        ps2[:, 0:nw],
                        lhsT=w2s[m][:, k * P : (k + 1) * P],
                        rhs=hb[b][k][:],
                        start=(k == 0),
                        stop=(k == KH - 1),
                    )
                ytt = wkp.tile([P, 512], f32, tag="ytt", bufs=2, name=f"ytt{b}_{m}")
                nc.scalar.add(
                    ytt[:, 0:nw], ps2[:, 0:nw], bb_sb[:, KH + m : KH + m + 1]
                )
                nc.vector.tensor_tensor(
                    out=ytt[:, 0:nw], in0=ytt[:, 0:nw],
                    in1=sbc[:, n0 : n0 + nw], op=AL.mult,
                )
                nc.sync.dma_start(yts[b].ap()[m], ytt[:, 0:nw])

            # router groups of block 0 double as the tail of the PE warmup;
            # each group's scale chunk follows it on the gpsimd queue
            for g in range(4):
                router_group(g)
                scale_chunk(g)

            for m in range(12):
                gemm1_slab(0, m)

            # remaining router groups mid-GEMM1(b0): xb1/xb2 have landed by
            # now, and the scale vector is complete well before the first
            # GEMM2 epilogue needs it
            for g in range(4, G):
                router_group(g)
                scale_chunk(g)

            for m in range(12, KH):
                gemm1_slab(0, m)

            for m in range(KD):
                gemm2_slab(0, m)
            for m in range(KH):
                gemm1_slab(1, m)
            # GEMM1 of the small block rides inside GEMM2(b1): its hb slabs
            # (and the m15 relu drain) are done well before GEMM2(b2)'s
            # short k-loop needs them, removing both block-boundary stalls
            for m in range(4):
                gemm2_slab(1, m)
            for m in range(KH):
                gemm1_slab(2, m)
            for m in range(4, KD):
                gemm2_slab(1, m)
            for m in range(KD):
                gemm2_slab(2, m)

    nc.compile()
    return nc


def get_module():
    if "nc" not in _cache:
        _cache["nc"] = _build()
    return _cache["nc"]


def _route(tok, Wr, br):
    """Host-side placement: which tokens go to which expert/core (argmax of
    the router). Only used for sharding; the device recomputes all values."""
    logits = tok @ Wr + br
    e = logits.argmax(-1)
    lists = []
    for c in range(NCORES):
        ids = np.nonzero(e == c)[0].astype(np.int32)
        assert len(ids) <= CAP, f"expert {c} overflows capacity: {len(ids)}"
        lists.append(ids)
    return lists


def make_in_maps(x, Wr, br, W1, b1, W2, b2):
    import ml_dtypes

    wdt = ml_dtypes.bfloat16 if BF16 else np.float32
    tok = np.ascontiguousarray(np.asarray(x, dtype=np.float32).reshape(T, D))
    Wr = np.ascontiguousarray(np.asarray(Wr, dtype=np.float32))
    br_ = np.asarray(br, dtype=np.float32).reshape(E)
    lists = _route(tok, Wr, br_)
    wrb = np.zeros((P, KD + 1, E), np.float32)
    wrb[:, :KD, :] = Wr.reshape(KD, P, E).transpose(1, 0, 2)
    wrb[0, KD, :] = br_
    wrb = np.ascontiguousarray(wrb.astype(wdt))
    in_maps = []
    for c in range(NCORES):
        ids = lists[c]
        n = len(ids)
        # dispatch + transpose on host (pure data movement): [D, CAP]
        xp = np.zeros((CAP, D), np.float32)
        xp[:n] = tok[ids]
        xT = xp.T.astype(wdt)  # [D, CAP]
        w1c = np.asarray(W1[c], dtype=np.float32)  # [D, H]
        w2c = np.asarray(W2[c], dtype=np.float32)  # [H, D]
        # slab layout [m, p, k*128+q]: lhsT chunk (k, m)[p, q] = W[128k+p, 128m+q]
        w1tc = np.ascontiguousarray(
            w1c.reshape(KD, P, KH, P).transpose(2, 1, 0, 3).reshape(KH, P, D)
        ).astype(wdt)
        w2tc = np.ascontiguousarray(
            w2c.reshape(KH, P, KD, P).transpose(2, 1, 0, 3).reshape(KD, P, H)
        ).astype(wdt)
        bbc = np.zeros((P, KH + KD), np.float32)
        bbc[:, :KH] = np.asarray(b1[c], dtype=np.float32).reshape(KH, P).T
        bbc[:, KH:] = np.asarray(b2[c], dtype=np.float32).reshape(KD, P).T
        m = {
            "wrb": wrb,
            "bb": np.ascontiguousarray(bbc),
            "w1t": w1tc,
            "w2t": w2tc,
        }
        for b, (n0, nw) in enumerate(NB):
            m[f"xt{b}"] = np.ascontiguousarray(
                xT[:, n0 : n0 + nw].reshape(KD, P, nw)
            )
        in_maps.append(m)
    return in_maps, lists


def combine(results, lists, x_shape):
    out = np.zeros((T, D), dtype=np.float32)
    for c in range(NCORES):
        n = len(lists[c])
        yT = np.concatenate(
            [
                np.asarray(results[c][f"yt{b}"]).astype(np.float32).reshape(D, nw)
                for b, (n0, nw) in enumerate(NB)
            ],
            axis=1,
        )  # [D, CAP]
        out[lists[c]] = yT[:, :n].T
    return out.reshape(x_shape)


def _unwedge_devices_once():
    # best-effort: clear any wedged state on the axon-tunneled NeuronCores
    # left behind by a previous crashed process
    if _cache.get("reset_done"):
        return
    _cache["reset_done"] = True
    try:
        import ctypes
        import jax

        jax.devices()
        lib = ctypes.CDLL("/opt/axon/libaxon_pjrt.so")
        lib.axon_reset.restype = ctypes.c_int64
        lib.axon_reset()
    except Exception:
        pass


def kernel(x, Wr, br, W1, b1, W2, b2):
    from concourse.bass_utils import run_bass_kernel_spmd

    _unwedge_devices_once()
    nc = get_module()
    in_maps, lists = make_in_maps(x, Wr, br, W1, b1, W2, b2)
    res = run_bass_kernel_spmd(nc, in_maps, core_ids=list(range(NCORES)))
    return combine(res.results, lists, np.asarray(x).shape)
